# revision 3
# baseline (speedup 1.0000x reference)
"""Trainium2 Bass kernel for a CPPN-style dense MLP forward pass.

Network (per pixel): 11 -> [32 x 23 tanh layers] -> 3 sigmoid.
  h = tanh(x @ W1.T); 22x: h = tanh(h @ Whid[l].T); out = sigmoid(h @ Wout.T)

Full inputs:  x [4194304, 11] f32, W1 [32, 11], Whid [22, 32, 32], Wout [3, 32]
Full output:  [4194304, 3] f32

Strategy: pure data parallel over 8 NeuronCores (pixels split 8 ways,
weights replicated).  Per core the kernel is activation-throughput bound
(ScalarE runs tanh at 1 elem/cycle/lane and nothing else has a tanh), so
on top of the baseline layout (16-way tile_position-packed 32x32 fp32
matmuls feeding [128, 2048] PSUM halves, A/B supertile ping-pong) this
version adds three things:

1. DVE tanh assist: the VectorE computes tanh for the last `DVE_W`
   columns of every layer's PSUM half via a clamped odd rational
   t*N(t^2)/D(t^2) (t = clamp(x, +-6)), evaluated in 4 DVE instructions:
   two custom fused ops (numerator/denominator polynomial, 8 ALU stages
   each), reciprocal_approx_fast (~51 ULP), and a tensor_mul.  Max abs
   error 2e-5 over all of R -- ~25x below what this chaotic 24-layer
   net can absorb while staying under the 2e-2 harness gate.  ScalarE
   and VectorE run concurrently on disjoint column ranges, raising
   total tanh throughput by ~18%.

2. Output free-slot rotation: layer k writes its output subtile for
   input partition-group u at free slot (u + k) & 3 instead of u.  The
   rotations cancel mod 4 over the 24 layers (final layout = baseline)
   but rotate WHICH pixels land in the DVE-assisted columns, so the
   small rational-approximation error is spread across subtiles
   instead of compounding on the same pixels for all 23 tanh layers.

3. Prefetched I/O + compact sigmoid: input DMAs are issued two pairs
   ahead and input transposes one pair ahead so ScalarE never waits at
   a pair boundary; the final layer transposes the logits first (DVE)
   and runs sigmoid only on the 3 valid output features per 32-feature
   group ([128, 192] instead of [128, 2048] -- 5.5x less ScalarE work).

I/O avoids small-packet DMA death as in the baseline: x is loaded
pixel-major in 44B chunks and block-transposed feature-major on the
VectorE (32x32 STREAM_TRANSPOSE); the output is block-transposed back
so the store scatters 12B/pixel chunks with a 32-row outer dim.  All
DMAs are issued from SyncE.  Matmuls are full fp32 (2-pass LOW/HIGH);
reduced matmul precision (f32r) measured ~0.14 L2 error -- unusable.
"""

import os
import sys

if "/opt/trn_rl_repo" not in sys.path:
    sys.path.insert(0, "/opt/trn_rl_repo")

import numpy as np

N_CORES = 8
N_PIX = 4194304
P_CORE = N_PIX // N_CORES      # 524288 pixels per core
D_IN = 11
D_H = 32
N_LAYERS = 24                  # 1 input + 22 hidden + 1 output
F = 512                        # pixels per tile (one PSUM bank of fp32)
ST_PIX = 16 * F                # 8192 pixels per supertile
N_ST = P_CORE // ST_PIX        # 64 supertiles per core
N_PAIRS = N_ST // 2            # 32 interleaved supertile pairs

# tanh(x) ~ t*(1 + s*(TN1 + s*TN2)) / (1 + q*(TD1 + q*(TD2 + q*TD3)))
# with t = clamp(x, +-TC), s = t*t, q = min(x*x, TC*TC).  Minimax fit on
# [0, 6] + saturation tail; max abs err 1.97e-5 in exact fp32 eval.
TC = 6.0
TN1 = 0.1148434653878212
TN2 = 0.0015149589162319899
TD1 = 0.4481341242790222
TD2 = 0.017614463344216347
TD3 = 5.6287932238774374e-05

W_COLS = N_LAYERS * 32 + 32    # weight tile + 32 cols of broadcast consts
CONST_COL = N_LAYERS * 32      # column holding TD3 on every partition

_BUILD_CACHE = {}
_DVE_OPS = {}


def _register_dve_ops():
    """Register the two fused tanh-rational DVE ops in dve_ops.OPS (the
    designed runtime-extension path: tables are generated per-NEFF from
    this registry and shipped via HLO frontend attributes)."""
    if _DVE_OPS:
        return _DVE_OPS
    import concourse.dve_ops as dve_ops
    from concourse.dve_ops import DveOp
    from concourse.dve_spec import (
        C0,
        C1,
        C2,
        C3,
        One,
        Spec,
        Src0,
        Zero,
        _spill_c3_to_src1,
        lower,
        maxx,
        minn,
        sq,
    )
    from concourse.dve_uop import DveOpSpec

    def _ref_p(in0, in1, s0, s1, imm2):
        t = np.clip(in0.astype(np.float32), -s0, s0)
        s = t * t
        return (t * (1.0 + s * (s1 + s * imm2))).astype(np.float32)

    t = maxx(minn(Src0, C0), Zero - C0)
    s = sq(t)
    body_p = t * (One + s * (C1 + s * C2))
    spec_p = Spec(body=body_p, reference=_ref_p)

    def _ref_d(in0, in1, s0, s1, imm2):
        q = np.minimum(in0.astype(np.float32) * in0, s0)
        return (1.0 + q * (s1 + q * (imm2 + q * in1))).astype(np.float32)

    q = minn(sq(Src0), C0)
    body_d = One + q * (C1 + q * (C2 + q * C3))
    spec_d = Spec(body=_spill_c3_to_src1(body_d), reference=_ref_d)

    made = {}
    for name, spec in (("TANH_P_ANT", spec_p), ("TANH_D_ANT", spec_d)):
        existing = {op.name: op for op in dve_ops.OPS}
        if name in existing:
            made[name] = existing[name]
            continue
        opcode = max(dve_ops._SUB_OPCODE_FOR_NAME.values()) + 1
        rd1 = name == "TANH_D_ANT"
        sha = {
            ver: DveOpSpec(
                name=name, opcode=opcode, uops=lower(spec, ver=ver), rd1_en=rd1
            ).sha(ver)
            for ver in ("v3", "v4")
        }
        op = DveOp(name, spec, subdim=False, uops_sha=sha)
        dve_ops.OPS.append(op)
        dve_ops._SUB_OPCODE_FOR_NAME[name] = opcode
        dve_ops.CUSTOM_DVE_SPECS[name] = spec
        made[name] = op
    _DVE_OPS.update(made)
    return _DVE_OPS


def _build(n_pairs, dve_w):
    """Build + bass-compile the per-core program. Returns the Bacc object."""
    import concourse.bass as bass  # noqa: F401
    import concourse.tile as tile
    from concourse import bacc, mybir
    from contextlib import ExitStack

    ops = _register_dve_ops()
    f32 = mybir.dt.float32
    Tanh = mybir.ActivationFunctionType.Tanh
    Sigmoid = mybir.ActivationFunctionType.Sigmoid

    nc = bacc.Bacc(
        "TRN2", target_bir_lowering=False, debug=False, num_devices=N_CORES
    )
    x_ap = nc.dram_tensor("x", [P_CORE, D_IN], f32, kind="ExternalInput").ap()
    w_ap = nc.dram_tensor("w", [128, W_COLS], f32, kind="ExternalInput").ap()
    o_ap = nc.dram_tensor("o", [P_CORE, 3], f32, kind="ExternalOutput").ap()

    act_w = 2048 - dve_w           # columns handled by ScalarE per layer

    with tile.TileContext(nc) as tc, ExitStack() as ctx:
        wp = ctx.enter_context(tc.tile_pool(name="wp", bufs=1))
        xrp = ctx.enter_context(tc.tile_pool(name="xrp", bufs=4))
        xp = ctx.enter_context(tc.tile_pool(name="xp", bufs=4))
        hp = ctx.enter_context(tc.tile_pool(name="hp", bufs=4))
        sp = ctx.enter_context(tc.tile_pool(name="sp", bufs=6))
        vp = ctx.enter_context(tc.tile_pool(name="vp", bufs=4))
        pp = ctx.enter_context(tc.tile_pool(name="pp", bufs=2, space="PSUM"))

        Wf = wp.tile([128, W_COLS], f32)
        nc.sync.dma_start(Wf[:], w_ap[:])
        d3c = Wf[:, CONST_COL : CONST_COL + 1]

        def load_dma(s, eng):
            # Stage 1: pixel-major load, 44B contiguous chunks per pixel row,
            # laid out so that 32x32 block-transpose yields feature-major
            # tiles: XR[32u+p, 32c+f] = x[s*8192 + u*2048 + 32c + p, f].
            XR = xrp.tile([128, 2048], f32)
            for u in range(4):
                p0 = s * ST_PIX + u * 2048
                eng.dma_start(
                    XR[32 * u : 32 * u + 32, :].rearrange(
                        "p (c f) -> p c f", c=64, f=32
                    )[:, :, 0:D_IN],
                    x_ap[p0 : p0 + 2048, :].rearrange("(c p) f -> p c f", c=64, p=32),
                )
            return XR

        def load_transpose(XR):
            # Stage 2: DVE 32x32 block transpose -> X[32u+f, 32c+p].
            X = xp.tile([128, 2048], f32)
            nc.vector.transpose(X[:], XR[:])
            return X

        def layer(H, k):
            """One layer for one supertile: 16 packed matmuls, then tanh on
            ScalarE (cols [0, act_w)) + VectorE rational (cols [act_w, 2048)).
            Output free-slot rotation: input pgroup u writes fslot (u+k)&3."""
            Kd = D_IN if k == 0 else 32
            last = k == N_LAYERS - 1
            rot = k & 3
            P_ = pp.tile([128, 2048], f32)
            # fp32 16-way tile-position packing; iterate so consecutive
            # matmuls land on different PE row groups (LDWEIGHTS only
            # pulls ahead of in-flight MMs when row_grp differs).
            ab = [(a, b) for b in range(4) for a in range(4)]
            if k % 2 == 1:
                ab = [(a, b) for a in range(4) for b in range(4)]
            for a, b in ab:
                u, v = (a, b) if k % 2 == 0 else (b, a)
                fs = (u + rot) & 3
                nc.tensor.matmul(
                    P_[32 * v : 32 * v + 32, 512 * fs : 512 * fs + 512],
                    lhsT=Wf[32 * u : 32 * u + Kd, 32 * k : 32 * k + 32],
                    rhs=H[32 * u : 32 * u + Kd, 512 * v : 512 * v + 512],
                    start=True,
                    stop=True,
                    tile_position=(32 * u, 32 * v),
                )
            if last:
                return P_
            Hn = hp.tile([128, 2048], f32)
            if dve_w > 0:
                nc.scalar.activation(Hn[:, 0:act_w], P_[:, 0:act_w], Tanh)
                xs = P_[:, act_w:2048]
                Pn = vp.tile([128, dve_w], f32)
                Dn = vp.tile([128, dve_w], f32)
                Rn = vp.tile([128, dve_w], f32)
                nc.vector._custom_dve(
                    ops["TANH_P_ANT"], out=Pn[:], in0=xs, s0=TC, s1=TN1, imm2=TN2
                )
                nc.vector._custom_dve(
                    ops["TANH_D_ANT"],
                    out=Dn[:],
                    in0=xs,
                    in1=d3c,
                    s0=TC * TC,
                    s1=TD1,
                    imm2=TD2,
                )
                nc.vector.reciprocal_approx_fast(out=Rn[:], in_=Dn[:])
                nc.vector.tensor_mul(Hn[:, act_w:2048], Pn[:], Rn[:])
            else:
                nc.scalar.activation(Hn[:], P_[:], Tanh)
            return Hn

        def store_out(s, P_, eng):
            # Block-transpose the logits back to pixel-major, sigmoid only
            # the 3 valid features per 32-group into a compact [128, 192]
            # tile, then scatter 12B/pixel chunks with a 32-row outer dim.
            # SR[32a+p, 32c+f] = P_[32a+f, 32c+p] = logit f of pixel
            # s*8192 + a*2048 + 32c + p.
            SR = sp.tile([128, 2048], f32)
            nc.vector.transpose(SR[:], P_[:])
            SO = sp.tile([128, 192], f32)
            nc.scalar.activation(
                SO[:].rearrange("p (c f) -> p c f", c=64, f=3),
                SR[:].rearrange("p (c f) -> p c f", c=64, f=32)[:, :, 0:3],
                Sigmoid,
            )
            for a in range(4):
                p0 = s * ST_PIX + a * 2048
                eng.dma_start(
                    o_ap[p0 : p0 + 2048, :].rearrange("(c p) f -> p c f", c=64, p=32),
                    SO[32 * a : 32 * a + 32, :].rearrange(
                        "p (c f) -> p c f", c=64, f=3
                    ),
                )

        # Software pipeline: DMAs two pairs ahead, transposes one pair
        # ahead, so ScalarE never stalls at a pair boundary.
        XRs = {}
        Xs = {}
        for p in range(min(2, n_pairs)):
            XRs[2 * p] = load_dma(2 * p, nc.sync)
            XRs[2 * p + 1] = load_dma(2 * p + 1, nc.sync)
        Xs[0] = load_transpose(XRs.pop(0))
        Xs[1] = load_transpose(XRs.pop(1))

        for pair in range(n_pairs):
            sA, sB = 2 * pair, 2 * pair + 1
            if pair + 2 < n_pairs:
                XRs[2 * (pair + 2)] = load_dma(2 * (pair + 2), nc.sync)
                XRs[2 * (pair + 2) + 1] = load_dma(2 * (pair + 2) + 1, nc.sync)
            if pair + 1 < n_pairs:
                Xs[2 * (pair + 1)] = load_transpose(XRs.pop(2 * (pair + 1)))
                Xs[2 * (pair + 1) + 1] = load_transpose(XRs.pop(2 * (pair + 1) + 1))
            HA, HB = Xs.pop(sA), Xs.pop(sB)
            # Interleave the two streams layer-by-layer so the PSUM pool's
            # two slots ping-pong A/B and ACT never waits on the PE.
            for k in range(N_LAYERS):
                HA = layer(HA, k)
                HB = layer(HB, k)
            store_out(sA, HA, nc.sync)
            store_out(sB, HB, nc.sync)

    nc.compile()
    return nc


def _get_program(n_pairs, dve_w):
    key = (n_pairs, dve_w)
    if key not in _BUILD_CACHE:
        _BUILD_CACHE[key] = _build(n_pairs, dve_w)
    return _BUILD_CACHE[key]


def _pack_weights(W1, Whid, Wout):
    """[128, W_COLS]: per partition-group u, column block l*32 holds W_l.T;
    column CONST_COL holds TD3 broadcast."""
    WT = np.zeros((N_LAYERS, 32, 32), np.float32)
    WT[0, :D_IN, :] = np.asarray(W1, np.float32).T
    WT[1:23] = np.transpose(np.asarray(Whid, np.float32), (0, 2, 1))
    WT[23, :, :3] = np.asarray(Wout, np.float32).T
    Wh = np.zeros((128, W_COLS), np.float32)
    blocks = WT.transpose(1, 0, 2).reshape(32, N_LAYERS * 32)
    for u in range(4):
        Wh[32 * u : 32 * u + 32, : N_LAYERS * 32] = blocks
    Wh[:, CONST_COL] = TD3
    return Wh


def _run(x, W1, Whid, Wout, trace=False, n_pairs=None, **spmd_kwargs):
    from concourse.bass_utils import run_bass_kernel_spmd

    if n_pairs is None:
        n_pairs = int(os.environ.get("BASSK_PAIRS", N_PAIRS))
    dve_w = int(os.environ.get("BASSK_DVEW", 224))
    nc = _get_program(n_pairs, dve_w)

    x = np.ascontiguousarray(np.asarray(x, np.float32))
    assert x.shape == (N_PIX, D_IN), x.shape
    Wh = _pack_weights(W1, Whid, Wout)

    in_maps = [
        {"x": x[i * P_CORE : (i + 1) * P_CORE], "w": Wh}
        for i in range(N_CORES)
    ]
    res = run_bass_kernel_spmd(
        nc, in_maps, list(range(N_CORES)), trace=trace, **spmd_kwargs
    )
    out = np.concatenate([res.results[i]["o"] for i in range(N_CORES)], axis=0)
    return out, res


def kernel(x, W1, Whid, Wout):
    out, _ = _run(x, W1, Whid, Wout)
    return out
